# revision 1
# baseline (speedup 1.0000x reference)
"""Trainium2 Bass kernel for nn_BiLinAntisymmetricFunc.

Math: out[b,n] = g(x1[b,n]) - g(x2[b,n]) + sum_k alpha_k * x1^T (U_k V_k^T - V_k U_k^T) x2

The bilinear part collapses: with S = sum_k alpha_k (U_k V_k^T - V_k U_k^T)
(a precomputable [D,D] weight matrix), bili = x1^T S x2 per row. Using
antisymmetry, bili = rowsum(x1 . (x2 @ (-S))). That is ONE [N,D]@[D,D]
matmul instead of four [N,D]@[D,512] projections (2x fewer FLOPs).

Sharding: data-parallel over the 65536 rows (B*NR) -> 8 cores x 8192 rows.
Each core receives x1 (row-major), x1^T and x2^T (d-major, host-pretransposed
so no on-chip transposes are needed), plus replicated weights.

Per 256-row pair of 128-row tiles, on device:
  - h1^T = relu(W1^T x^T + b1) for x1 and x2 (K=D contraction, 8 chunk matmuls,
    W1 stationary, x^T moving with N=256 for full-rate f32r)
  - h2^T = relu(W2^T h1^T + b2); gdiff = (stacked h2^T)^T @ [W3; -W3]  -> [128,1]
  - P = x2 @ (-S): lhsT = x2^T chunks (stationary), rhs = S chunks, fp32 PSUM
  - bili+gdiff via DVE tensor_tensor_reduce: out=(x1.P), accum=rowsum with
    initial value chained (gdiff -> seg0 acc -> seg1 acc = final row values)
"""

import os

import numpy as np

D, K, RANK = 1024, 8, 64
B, NR = 16, 4096
NCORES = 8
TOTAL_ROWS = B * NR
ROWS = TOTAL_ROWS // NCORES  # 8192 rows per core

# matmul dtype for the PE: "f32r" (full-rate fp32-storage), "f32" (quarter
# rate, full precision) or "bf16" (full rate, casts inputs on host)
MM_DT = os.environ.get("BILIN_MM_DT", "f32r")

_PROG_CACHE = {}


def _build_program(rows, mm_dt, variant=None, reps=1):
    # variant: comma-separated debug switches: "nomlp" (skip tiny MLP, g=0),
    # "noapscalar" (TTR initial value from SBUF copy instead of PSUM AP)
    variant = variant if variant is not None else os.environ.get("BILIN_VARIANT", "")
    nomlp = "nomlp" in variant
    noapscalar = "noapscalar" in variant
    dmaonly = "dmaonly" in variant
    computeonly = "computeonly" in variant
    if dmaonly:
        nomlp = True
    import concourse.bacc as bacc
    import concourse.bass as bass
    import concourse.mybir as mybir
    import concourse.tile as tile

    f32 = mybir.dt.float32
    bf16 = mybir.dt.bfloat16
    if mm_dt == "f32r":
        mdt = mybir.dt.float32r  # big matmuls: full-rate, fp32 storage
        hdt = bf16  # tiny MLP h-chain (negligible output contribution)
    elif mm_dt == "bf16":
        mdt = bf16
        hdt = bf16
    else:
        mdt = f32
        hdt = f32

    nc = bacc.Bacc("TRN2", target_bir_lowering=False, debug=False)

    NCHUNK = D // 128  # 8 contraction chunks
    PAIR = 512  # rows per group (four 128-row tiles)
    npairs = rows // PAIR

    # packed layouts: exact SBUF tile images for long-contiguous DMA runs
    x1_d = nc.dram_tensor("x1", [rows, D], f32, kind="ExternalInput")
    x1t_d = nc.dram_tensor("x1t", [npairs, 128, NCHUNK * PAIR], mdt, kind="ExternalInput")
    x2t_d = nc.dram_tensor("x2t", [npairs, 128, NCHUNK * PAIR], mdt, kind="ExternalInput")
    s_d = nc.dram_tensor("s", [128, NCHUNK * D], mdt, kind="ExternalInput")  # packed -S
    w1_d = nc.dram_tensor("w1", [128, NCHUNK * K], mdt, kind="ExternalInput")
    b1_d = nc.dram_tensor("b1", [K, 1], f32, kind="ExternalInput")
    w2_d = nc.dram_tensor("w2", [K, K], hdt, kind="ExternalInput")
    b2_d = nc.dram_tensor("b2", [K, 1], f32, kind="ExternalInput")
    w3p_d = nc.dram_tensor("w3p", [K, 1], hdt, kind="ExternalInput")
    w3n_d = nc.dram_tensor("w3n", [K, 1], hdt, kind="ExternalInput")
    out_d = nc.dram_tensor("out", [rows, 1], f32, kind="ExternalOutput")

    relu = mybir.ActivationFunctionType.Relu
    mult = mybir.AluOpType.mult
    add = mybir.AluOpType.add

    with tile.TileContext(nc) as tc:
        with (
            tc.tile_pool(name="const", bufs=1) as cpool,
            tc.tile_pool(name="x1p", bufs=4) as x1pool,
            tc.tile_pool(name="xtp", bufs=3) as xtpool,
            tc.tile_pool(name="hsb", bufs=2) as hpool,
            tc.tile_pool(name="scr", bufs=2) as scrpool,
            tc.tile_pool(name="acc", bufs=6) as accpool,
            tc.tile_pool(name="pp", bufs=4, space="PSUM") as ppool,
            tc.tile_pool(name="hp", bufs=2, space="PSUM") as hppool,
            tc.tile_pool(name="gp", bufs=2, space="PSUM") as gppool,
        ):
            # ---- resident constants ----
            s_sb = cpool.tile([128, NCHUNK * D], mdt)  # chunk c at cols [c*D,(c+1)*D)
            nc.sync.dma_start(s_sb[:], s_d[:, :])
            w1_sb = cpool.tile([128, NCHUNK * K], mdt)  # chunk c at cols [c*K,(c+1)*K)
            nc.sync.dma_start(w1_sb[:], w1_d[:, :])
            w2_sb = cpool.tile([K, K], hdt)
            nc.sync.dma_start(w2_sb[:], w2_d[:, :])
            w3p_sb = cpool.tile([K, 1], hdt)
            nc.sync.dma_start(w3p_sb[:], w3p_d[:, :])
            w3n_sb = cpool.tile([K, 1], hdt)
            nc.sync.dma_start(w3n_sb[:], w3n_d[:, :])
            b1_sb = cpool.tile([K, 1], f32)
            nc.sync.dma_start(b1_sb[:], b1_d[:, :])
            b2_sb = cpool.tile([K, 1], f32)
            nc.sync.dma_start(b2_sb[:], b2_d[:, :])

            pre = {}
            if computeonly:
                pre["x1t"] = cpool.tile([128, NCHUNK * PAIR], mdt, name="pre_x1t")
                pre["x2t"] = cpool.tile([128, NCHUNK * PAIR], mdt, name="pre_x2t")
                nc.sync.dma_start(pre["x1t"][:], x1t_d[0, :, :])
                nc.sync.dma_start(pre["x2t"][:], x2t_d[0, :, :])
                pre["x1"] = []
                for i in range(PAIR // 128):
                    xt = cpool.tile([128, D], f32, name=f"pre_x1_{i}")
                    nc.sync.dma_start(xt[:], x1_d[i * 128 : (i + 1) * 128, :])
                    pre["x1"].append(xt)
            if dmaonly:
                zacc = cpool.tile([128, 1], f32, name="zacc")
                nc.vector.memset(zacc[:], 0.0)

            def emit_pair(p):
                pp = p % npairs
                r0 = pp * PAIR
                # ---- loads ----
                if computeonly:
                    x1t_t, x2t_t, x1_tiles = pre["x1t"], pre["x2t"], pre["x1"]
                else:
                    x1t_t = xtpool.tile([128, NCHUNK * PAIR], mdt, tag="x1t")
                    x2t_t = xtpool.tile([128, NCHUNK * PAIR], mdt, tag="x2t")
                    nc.sync.dma_start(x1t_t[:], x1t_d[pp, :, :])
                    nc.sync.dma_start(x2t_t[:], x2t_d[pp, :, :])
                    x1_tiles = []
                    for i in range(PAIR // 128):
                        xt = x1pool.tile([128, D], f32, tag="x1")
                        nc.sync.dma_start(xt[:], x1_d[r0 + i * 128 : r0 + (i + 1) * 128, :])
                        x1_tiles.append(xt)
                if dmaonly:
                    for i in range(PAIR // 128):
                        nc.sync.dma_start(
                            out_d[r0 + i * 128 : r0 + (i + 1) * 128, :], zacc[:]
                        )
                    return

                # ---- tiny MLP (transposed form; k on partitions) ----
                h2sb = []
                if nomlp:
                    h2sb = None
                el_guard = None
                for j, xt_t in enumerate(() if nomlp else (x1t_t, x2t_t)):
                    h1ps = hppool.tile([K, PAIR], f32, tag="hps", name=f"h1ps{p}_{j}")
                    for c in range(NCHUNK):
                        nc.tensor.matmul(
                            h1ps[:],
                            w1_sb[:, c * K : (c + 1) * K],
                            xt_t[:, c * PAIR : (c + 1) * PAIR],
                            start=(c == 0),
                            stop=(c == NCHUNK - 1),
                        )
                    h1sb = hpool.tile([K, PAIR], hdt, tag="h1sb", name=f"h1sb{p}_{j}")
                    nc.scalar.activation(h1sb[:], h1ps[:], relu, bias=b1_sb[:])
                    h2ps = hppool.tile([K, PAIR], f32, tag="hps", name=f"h2ps{p}_{j}")
                    nc.tensor.matmul(h2ps[:], w2_sb[:], h1sb[:])
                    h2s = hpool.tile([K, PAIR], hdt, tag="h2sb", name=f"h2sb{p}_{j}")
                    nc.scalar.activation(h2s[:], h2ps[:], relu, bias=b2_sb[:])
                    h2sb.append(h2s)

                for i in range(PAIR // 128):
                    # gdiff = g(x1) - g(x2) for this 128-row tile, [128,1] PSUM
                    if nomlp:
                        gsrc = 0.0
                    else:
                        gps = gppool.tile([128, 1], f32, tag="g", name=f"g{p}_{i}")
                        nc.tensor.matmul(
                            gps[:],
                            h2sb[0][:, i * 128 : (i + 1) * 128],
                            w3p_sb[:],
                            start=True,
                            stop=False,
                        )
                        nc.tensor.matmul(
                            gps[:],
                            h2sb[1][:, i * 128 : (i + 1) * 128],
                            w3n_sb[:],
                            start=False,
                            stop=True,
                        )
                        if noapscalar:
                            gsb = accpool.tile([128, 1], f32, tag="acc", name=f"gsb{p}_{i}")
                            nc.scalar.copy(gsb[:], gps[:])
                            gsrc = gsb[:]
                        else:
                            gsrc = gps[:]
                    # P = x2 @ (-S) for this tile: two 512-col segments
                    pps = [
                        ppool.tile([128, 512], f32, tag="P", name=f"P{p}_{i}_{s_}")
                        for s_ in range(2)
                    ]
                    for c in range(NCHUNK):
                        lhs = x2t_t[:, c * PAIR + i * 128 : c * PAIR + (i + 1) * 128]
                        for seg in range(2):
                            nc.tensor.matmul(
                                pps[seg][:],
                                lhs,
                                s_sb[:, c * D + seg * 512 : c * D + (seg + 1) * 512],
                                start=(c == 0),
                                stop=(c == NCHUNK - 1),
                            )
                    # bili: fused mul + rowsum via scalar_tensor_tensor accum
                    acc0 = accpool.tile([128, 1], f32, tag="acc")
                    acc1 = accpool.tile([128, 1], f32, tag="acc")
                    fin = accpool.tile([128, 1], f32, tag="acc")
                    scr0 = scrpool.tile([128, 512], f32, tag="scr")
                    scr1 = scrpool.tile([128, 512], f32, tag="scr")
                    nc.vector.scalar_tensor_tensor(
                        scr0[:], x1_tiles[i][:, 0:512], 1.0, pps[0][:],
                        op0=mult, op1=mult, accum_out=acc0[:],
                    )
                    nc.vector.scalar_tensor_tensor(
                        scr1[:], x1_tiles[i][:, 512:1024], 1.0, pps[1][:],
                        op0=mult, op1=mult, accum_out=acc1[:],
                    )
                    nc.vector.tensor_add(fin[:], acc0[:], acc1[:])
                    if not nomlp:
                        fin2 = accpool.tile([128, 1], f32, tag="acc")
                        nc.vector.tensor_add(fin2[:], fin[:], gsrc)
                        fin = fin2
                    nc.sync.dma_start(
                        out_d[r0 + i * 128 : r0 + (i + 1) * 128, :], fin[:]
                    )

            if reps > 1:
                with tc.For_i(0, reps, 1):
                    for p in range(npairs):
                        emit_pair(p)
            else:
                for p in range(npairs):
                    emit_pair(p)
    nc.compile()
    return nc


def get_program(rows=ROWS, mm_dt=MM_DT):
    key = (rows, mm_dt)
    if key not in _PROG_CACHE:
        _PROG_CACHE[key] = _build_program(rows, mm_dt)
    return _PROG_CACHE[key]


def _pack_xt(x, npairs, PAIR, NCHUNK):
    """[rows, D] -> [npairs, 128, NCHUNK*PAIR]; [pair,p,c*PAIR+r] = x[pair*PAIR+r, c*128+p]."""
    return np.ascontiguousarray(
        x.reshape(npairs, PAIR, NCHUNK, 128)
        .transpose(0, 3, 2, 1)
        .reshape(npairs, 128, NCHUNK * PAIR)
    )


def prep_host(x1, x2, U, V, alpha, W1, b1, W2, b2, W3, b3, rows=ROWS, mm_dt=MM_DT):
    """Host-side prep: fold U,V,alpha into -S, stack W3, shard + transpose x."""
    f64 = np.float64
    Uf = np.asarray(U, f64).transpose(1, 0, 2).reshape(D, K * RANK)
    Vaf = (np.asarray(V, f64) * np.asarray(alpha, f64)[:, None, None])
    Vaf = Vaf.transpose(1, 0, 2).reshape(D, K * RANK)
    A = Uf @ Vaf.T
    s_use = (A.T - A)  # == -S ; bili = rowsum(x1 * (x2 @ s_use))

    import ml_dtypes

    mnp = np.dtype(ml_dtypes.bfloat16) if mm_dt == "bf16" else np.dtype(np.float32)
    hnp = np.dtype(np.float32) if mm_dt == "f32" else np.dtype(ml_dtypes.bfloat16)

    NCHUNK = D // 128
    PAIR = 512
    npairs = rows // PAIR
    # pack [D, N] -> [128, NCHUNK*N]: row c*128+p -> [p, c*N + n]
    s_use = np.ascontiguousarray(
        s_use.reshape(NCHUNK, 128, D).transpose(1, 0, 2).reshape(128, NCHUNK * D)
    ).astype(mnp)
    w1 = np.ascontiguousarray(
        np.asarray(W1, np.float64).reshape(NCHUNK, 128, K).transpose(1, 0, 2).reshape(128, NCHUNK * K)
    ).astype(mnp)
    w2 = np.ascontiguousarray(np.asarray(W2, np.float32)).astype(hnp)
    w3p = np.asarray(W3, f64).astype(hnp)
    w3n = (-np.asarray(W3, f64)).astype(hnp)
    b1c = np.ascontiguousarray(np.asarray(b1, np.float32).reshape(K, 1))
    b2c = np.ascontiguousarray(np.asarray(b2, np.float32).reshape(K, 1))

    x1f = np.asarray(x1, np.float32).reshape(TOTAL_ROWS, D)
    x2f = np.asarray(x2, np.float32).reshape(TOTAL_ROWS, D)
    ncores = TOTAL_ROWS // rows
    in_maps = []
    for c in range(ncores):
        sl = slice(c * rows, (c + 1) * rows)
        in_maps.append(
            {
                "x1": np.ascontiguousarray(x1f[sl]),
                "x1t": _pack_xt(x1f[sl], npairs, PAIR, NCHUNK).astype(mnp),
                "x2t": _pack_xt(x2f[sl], npairs, PAIR, NCHUNK).astype(mnp),
                "s": s_use,
                "w1": w1,
                "b1": b1c,
                "w2": w2,
                "b2": b2c,
                "w3p": w3p,
                "w3n": w3n,
            }
        )
    return in_maps


def run(inputs, trace=False, mm_dt=MM_DT):
    """Run on the 8 NeuronCores. Returns (full_output [B,NR] f32, BassKernelResults)."""
    from concourse.bass_utils import run_bass_kernel_spmd

    in_maps = prep_host(**inputs, rows=ROWS, mm_dt=mm_dt)
    nc = get_program(ROWS, mm_dt)
    res = run_bass_kernel_spmd(nc, in_maps, list(range(NCORES)), trace=trace)
    out = np.concatenate(
        [res.results[c]["out"].reshape(ROWS) for c in range(NCORES)]
    )
    return out.reshape(B, NR).astype(np.float32), res


def kernel(**inputs):
    out, _ = run(inputs, trace=False)
    return out



# revision 26
# speedup vs baseline: 1.3736x; 1.3736x over previous
"""Trainium2 Bass kernel for nn_BiLinAntisymmetricFunc.

Math: out[b,n] = g(x1[b,n]) - g(x2[b,n]) + sum_k alpha_k * x1^T (U_k V_k^T - V_k U_k^T) x2

The bilinear part collapses: with S = sum_k alpha_k (U_k V_k^T - V_k U_k^T)
(a precomputable [D,D] weight matrix), bili = x1^T S x2 per row. Using
antisymmetry, bili = rowsum(x1 . (x2 @ (-S))): ONE [N,D]@[D,D] matmul.

Sharding: data-parallel over the 65536 rows (B*NR) -> 8 cores x 8192 rows.

v3 design:
  - all x tensors in bf16 (half HBM traffic, full PE rate, FWL weight loads)
  - one packed DMA per 512-row pair (x1t | x2t | x1row concatenated: 3MB)
    -> ~17 large transfers per core instead of ~112 (DMA fixed cost is ~2us
    per serialized HWDGE transfer on this part)
  - h1 matmuls 2-way column-tiled (tile_position via out base partition 0/32
    for x1, 64/96 for x2) -> 2 concurrent 512-col streams; all 16 chunk
    matmuls accumulate into ONE PSUM bank using per-element has_written
    (single start=True on the first matmul)
  - h-chain software-pipelined one pair ahead: PE order per steady block p is
    [P-matmuls pair p] [h2 pair p+1] [h1 pair p+2] [g pair p+1], so ACT/DVE
    round trips never stall the PE
  - g(x1)-g(x2) via h2stack [16,512] and w3stack [W3;-W3]: one [16,128]x[16,1]
    matmul per 128-row tile into a shared [128,4] PSUM tile
  - P PSUM tile is [128,1024] (2 banks); one DVE scalar_tensor_tensor per
    128-row tile does x1.P with rowsum accum -> [128,1]
  - outputs collect into a [128,64] SBUF buffer, one 32KB store per core;
    out[p,t] = row t*128+p (host transposes)
"""

import os

import numpy as np

D, K, RANK = 1024, 8, 64
B, NR = 16, 4096
NCORES = 8
TOTAL_ROWS = B * NR
ROWS = TOTAL_ROWS // NCORES  # 8192 rows per core

MM_DT = os.environ.get("BILIN_MM_DT", "bf16")

_PROG_CACHE = {}

NCHUNK = D // 128  # 8 contraction chunks
PAIR = 512  # rows per group (four 128-row tiles)
NTILE = PAIR // 128
XT_COLS = NCHUNK * PAIR  # 4096
XROW_COLS = NTILE * D  # 4096
XALL_COLS = 2 * XT_COLS + XROW_COLS  # x1t | x2t | x1row


def _build_program(rows, mm_dt, variant=None, reps=1):
    # variant switches: "dmaonly" (skip compute), "computeonly" (no per-pair DMA)
    variant = variant if variant is not None else os.environ.get("BILIN_VARIANT", "")
    dmaonly = "dmaonly" in variant
    computeonly = "computeonly" in variant
    import concourse.bacc as bacc
    import concourse.mybir as mybir
    import concourse.tile as tile

    f32 = mybir.dt.float32
    bf16 = mybir.dt.bfloat16
    if mm_dt == "bf16":
        mdt = bf16
    else:
        mdt = mybir.dt.float32r

    nc = bacc.Bacc("TRN2", target_bir_lowering=False, debug=False)

    npairs = rows // PAIR
    ntiles = rows // 128

    xall_d = nc.dram_tensor("xall", [npairs, 128, XALL_COLS], mdt, kind="ExternalInput")
    s_d = nc.dram_tensor("s", [128, NCHUNK * D], mdt, kind="ExternalInput")  # packed -S
    w1_d = nc.dram_tensor("w1", [128, NCHUNK * K], mdt, kind="ExternalInput")
    b1_d = nc.dram_tensor("b1", [K, 1], f32, kind="ExternalInput")
    w2_d = nc.dram_tensor("w2", [K, K], mdt, kind="ExternalInput")
    b2_d = nc.dram_tensor("b2", [K, 1], f32, kind="ExternalInput")
    w3p_d = nc.dram_tensor("w3p", [K, 1], mdt, kind="ExternalInput")  # W3
    w3n_d = nc.dram_tensor("w3n", [K, 1], mdt, kind="ExternalInput")  # -W3
    # out[p, t] = result row t*128+p  (host transposes)
    out_d = nc.dram_tensor("out", [128, ntiles], f32, kind="ExternalOutput")

    relu = mybir.ActivationFunctionType.Relu
    mult = mybir.AluOpType.mult

    with tile.TileContext(nc) as tc:
        with (
            tc.tile_pool(name="const", bufs=1) as cpool,
            tc.tile_pool(name="xall", bufs=4) as xpool,
            tc.tile_pool(name="hsb", bufs=2) as hpool,
            tc.tile_pool(name="scr", bufs=2) as scrpool,
            tc.tile_pool(name="acc", bufs=8) as accpool,
            tc.tile_pool(name="pp", bufs=2, space="PSUM") as ppool,
            tc.tile_pool(name="hp1a", bufs=1, space="PSUM") as hp1apool,
            tc.tile_pool(name="hp1b", bufs=1, space="PSUM") as hp1bpool,
            tc.tile_pool(name="hp2", bufs=1, space="PSUM") as hp2pool,
            tc.tile_pool(name="gp", bufs=1, space="PSUM") as gppool,
        ):
            # ---- resident constants ----
            s_sb = cpool.tile([128, NCHUNK * D], mdt)  # chunk c at cols [c*D,(c+1)*D)
            nc.sync.dma_start(s_sb[:], s_d[:, :])
            w1_sb = cpool.tile([128, NCHUNK * K], mdt)  # chunk c at cols [c*K,(c+1)*K)
            nc.sync.dma_start(w1_sb[:], w1_d[:, :])
            w2_sb = cpool.tile([K, K], mdt)
            nc.sync.dma_start(w2_sb[:], w2_d[:, :])
            w3p_sb = cpool.tile([K, 1], mdt)
            nc.sync.dma_start(w3p_sb[:], w3p_d[:, :])
            w3n_sb = cpool.tile([K, 1], mdt)
            nc.sync.dma_start(w3n_sb[:], w3n_d[:, :])
            b1_sb = cpool.tile([K, 1], f32)
            nc.sync.dma_start(b1_sb[:], b1_d[:, :])
            b2_sb = cpool.tile([K, 1], f32)
            nc.sync.dma_start(b2_sb[:], b2_d[:, :])
            outbuf = cpool.tile([128, ntiles], f32, name="outbuf")
            if dmaonly:
                nc.vector.memset(outbuf[:], 0.0)

            # pipeline state (trace-time): pair index -> live tile objects
            xall = {}
            h1sb = {}
            h2stack = {}
            gps = {}

            pre_xall = None
            if computeonly:
                pre_xall = cpool.tile([128, XALL_COLS], mdt, name="pre_xall")
                nc.sync.dma_start(pre_xall[:], xall_d[0, :, :])

            if computeonly:
                for j in range(npairs):
                    xall[j] = pre_xall

            def load(j, eng=None):
                if computeonly:
                    return
                t = xpool.tile([128, XALL_COLS], mdt, tag="xall")
                (eng or nc.sync).dma_start(t[:], xall_d[j % npairs, :, :])
                xall[j % npairs] = t

            def x1t(j):
                return xall[j][:, 0:XT_COLS]

            def x2t(j):
                return xall[j][:, XT_COLS : 2 * XT_COLS]

            def x1row(j):
                return xall[j][:, 2 * XT_COLS :]

            def emit_h1(j):
                """h1^T for both x of pair j: 16 chunk-matmuls, 2-way
                col-tiled across two PSUM banks (x1 @ base 0 of bank A, x2 @
                base 64 of bank B; interleaved for stream concurrency). Each
                bank has its own clean accumulation group — a shared bank
                would race its bank-wide has_written clear against the
                concurrent other group's first writes.
                ACT relus (one aligned PSUM read each) -> h1sb [8,512] per x."""
                jj = j % npairs
                h1pa = hp1apool.tile([128, PAIR], f32, tag="h1pa", name=f"h1pa{j}")
                h1pb = hp1bpool.tile([128, PAIR], f32, tag="h1pb", name=f"h1pb{j}")
                parts = (h1pa, h1pb)
                xts = (x1t(jj), x2t(jj))
                for c in range(NCHUNK):
                    for xi in range(2):
                        grp = 64 * xi
                        nc.tensor.matmul(
                            parts[xi][grp : grp + K, :],
                            w1_sb[:, c * K : (c + 1) * K],
                            xts[xi][:, c * PAIR : (c + 1) * PAIR],
                            start=(c == 0),
                            stop=(c == NCHUNK - 1),
                            tile_position=(0, grp),
                        )
                # two separate base-0 tiles: the h2 matmul contracts over
                # partitions, so rhs must share lhsT's (w2) partition range
                hs1 = hpool.tile([K, PAIR], mdt, tag="h1sb", name=f"h1sb{j}_0")
                hs2 = hpool.tile([K, PAIR], mdt, tag="h1sb", name=f"h1sb{j}_1")
                for xi, hs in enumerate((hs1, hs2)):
                    nc.scalar.activation(
                        hs[:, :], parts[xi][64 * xi : 64 * xi + K, :], relu, bias=b1_sb[:]
                    )
                h1sb[j % npairs] = (hs1, hs2)

            def emit_h2(j):
                """h2^T for pair j: 2 col-tiled matmuls into one bank at
                partitions {0-7},{32-39}; ACT relu -> h2stack [16,512] bf16."""
                jj = j % npairs
                hs1, hs2 = h1sb[jj]
                h2ps = hp2pool.tile([128, PAIR], f32, tag="h2ps", name=f"h2p{j}")
                nc.tensor.matmul(h2ps[0:K, :], w2_sb[:], hs1[:, :], start=True, stop=True)
                nc.tensor.matmul(
                    h2ps[32 : 32 + K, :], w2_sb[:], hs2[:, :],
                    start=False, stop=True, skip_group_check=True,
                    tile_position=(0, 32),
                )
                h2s1 = hpool.tile([K, PAIR], mdt, tag="h2sb", name=f"h2sb{j}_0")
                h2s2 = hpool.tile([K, PAIR], mdt, tag="h2sb", name=f"h2sb{j}_1")
                nc.scalar.activation(h2s1[:, :], h2ps[0:K, :], relu, bias=b2_sb[:])
                nc.scalar.activation(h2s2[:, :], h2ps[32 : 32 + K, :], relu, bias=b2_sb[:])
                h2stack[jj] = (h2s1, h2s2)

            def emit_g(j):
                """gdiff for pair j: per 128-row tile, two [8,128]x[8,1]
                matmuls (g(x1) with W3, g(x2) with -W3) accumulating into one
                column of a shared [128,4] PSUM tile."""
                jj = j % npairs
                h2s1, h2s2 = h2stack[jj]
                g = gppool.tile([128, NTILE], f32, tag="g", name=f"g{j}")
                for i in range(NTILE):
                    nc.tensor.matmul(
                        g[:, i : i + 1],
                        h2s1[:, i * 128 : (i + 1) * 128],
                        w3p_sb[:],
                        start=(i == 0),
                        stop=False,
                        skip_group_check=True,
                    )
                    nc.tensor.matmul(
                        g[:, i : i + 1],
                        h2s2[:, i * 128 : (i + 1) * 128],
                        w3n_sb[:],
                        start=False,
                        stop=True,
                        skip_group_check=True,
                    )
                gps[jj] = g

            def emit_P(p):
                """P matmuls + rowsum TTR for all 4 tiles of pair p."""
                pp = p % npairs
                for i in range(NTILE):
                    pps = ppool.tile([128, 2 * 512], f32, tag="P", name=f"P{p}_{i}")
                    for c in range(NCHUNK):
                        lhs = x2t(pp)[:, c * PAIR + i * 128 : c * PAIR + (i + 1) * 128]
                        for seg in range(2):
                            nc.tensor.matmul(
                                pps[:, seg * 512 : (seg + 1) * 512],
                                lhs,
                                s_sb[:, c * D + seg * 512 : c * D + (seg + 1) * 512],
                                start=(c == 0),
                                stop=(c == NCHUNK - 1),
                            )
                    acc = accpool.tile([128, 1], f32, tag="acc")
                    scr = scrpool.tile([128, 2 * 512], f32, tag="scr")
                    nc.vector.scalar_tensor_tensor(
                        scr[:], x1row(pp)[:, i * D : (i + 1) * D], 1.0, pps[:],
                        op0=mult, op1=mult, accum_out=acc[:],
                    )
                    accs.append((p, i, acc))

            def emit_final(p):
                pp = p % npairs
                g = gps[pp]
                for (q, i, acc) in [a for a in accs if a[0] == p]:
                    nc.vector.tensor_add(
                        outbuf[:, pp * NTILE + i : pp * NTILE + i + 1],
                        acc[:],
                        g[:, i : i + 1],
                    )
                accs[:] = [a for a in accs if a[0] != p]

            accs = []

            PREFETCH = 3

            def emit_full():
                """One complete self-contained pass: prologue (first loads +
                h-chain for pairs 0,1, g for pair 0) then the pair blocks."""
                for j in range(min(PREFETCH, npairs)):
                    load(j)
                if not dmaonly:
                    emit_h1(0)
                    emit_h2(0)
                    emit_h1(1)
                    emit_g(0)
                for p in range(npairs):
                    jn = p + PREFETCH
                    if jn < npairs:
                        load(jn)
                    if dmaonly:
                        continue
                    emit_P(p)
                    # final(p) before g(p+1): gps is single-buffered, so the
                    # g matmuls of p+1 must WAR-wait on final(p)'s reads
                    emit_final(p)
                    if p + 1 < npairs:
                        emit_h2(p + 1)
                    if p + 2 < npairs:
                        emit_h1(p + 2)
                    if p + 1 < npairs:
                        emit_g(p + 1)
                nc.sync.dma_start(out_d[:, :], outbuf[:])

            if reps > 1:
                with tc.For_i(0, reps, 1):
                    emit_full()
            else:
                emit_full()
    nc.compile()
    return nc


def get_program(rows=ROWS, mm_dt=MM_DT):
    key = (rows, mm_dt)
    if key not in _PROG_CACHE:
        _PROG_CACHE[key] = _build_program(rows, mm_dt)
    return _PROG_CACHE[key]


def _pack_xt(x, npairs):
    """[rows, D] -> [npairs, 128, NCHUNK*PAIR]; [pair,p,c*PAIR+r] = x[pair*PAIR+r, c*128+p]."""
    return (
        x.reshape(npairs, PAIR, NCHUNK, 128)
        .transpose(0, 3, 2, 1)
        .reshape(npairs, 128, NCHUNK * PAIR)
    )


def _pack_xrow(x, npairs):
    """[rows, D] -> [npairs, 128, NTILE*D]; [pair,p,i*D+d] = x[pair*PAIR+i*128+p, d]."""
    return (
        x.reshape(npairs, NTILE, 128, D).transpose(0, 2, 1, 3).reshape(
            npairs, 128, NTILE * D
        )
    )


def prep_host(x1, x2, U, V, alpha, W1, b1, W2, b2, W3, b3, rows=ROWS, mm_dt=MM_DT):
    """Host-side prep: fold U,V,alpha into -S, stack W3, shard + pack x."""
    f64 = np.float64
    Uf = np.asarray(U, f64).transpose(1, 0, 2).reshape(D, K * RANK)
    Vaf = (np.asarray(V, f64) * np.asarray(alpha, f64)[:, None, None])
    Vaf = Vaf.transpose(1, 0, 2).reshape(D, K * RANK)
    A = Uf @ Vaf.T
    s_use = (A.T - A)  # == -S ; bili = rowsum(x1 * (x2 @ s_use))

    import ml_dtypes

    mnp = np.dtype(ml_dtypes.bfloat16) if mm_dt == "bf16" else np.dtype(np.float32)

    npairs = rows // PAIR
    # pack [D, N] -> [128, NCHUNK*N]: row c*128+p -> [p, c*N + n]
    s_pk = np.ascontiguousarray(
        s_use.reshape(NCHUNK, 128, D).transpose(1, 0, 2).reshape(128, NCHUNK * D)
    ).astype(mnp)
    w1 = np.ascontiguousarray(
        np.asarray(W1, np.float64).reshape(NCHUNK, 128, K).transpose(1, 0, 2).reshape(128, NCHUNK * K)
    ).astype(mnp)
    w2 = np.ascontiguousarray(np.asarray(W2, np.float32)).astype(mnp)
    w3p = np.asarray(W3, f64).astype(mnp)
    w3n = (-np.asarray(W3, f64)).astype(mnp)
    b1c = np.ascontiguousarray(np.asarray(b1, np.float32).reshape(K, 1))
    b2c = np.ascontiguousarray(np.asarray(b2, np.float32).reshape(K, 1))

    x1f = np.asarray(x1, np.float32).reshape(TOTAL_ROWS, D)
    x2f = np.asarray(x2, np.float32).reshape(TOTAL_ROWS, D)
    ncores = TOTAL_ROWS // rows
    in_maps = []
    for c in range(ncores):
        sl = slice(c * rows, (c + 1) * rows)
        xall = np.concatenate(
            [
                _pack_xt(x1f[sl], npairs),
                _pack_xt(x2f[sl], npairs),
                _pack_xrow(x1f[sl], npairs),
            ],
            axis=2,
        ).astype(mnp)
        in_maps.append(
            {
                "xall": np.ascontiguousarray(xall),
                "s": s_pk,
                "w1": w1,
                "b1": b1c,
                "w2": w2,
                "b2": b2c,
                "w3p": w3p,
                "w3n": w3n,
            }
        )
    return in_maps


def unshard_out(res_list, rows=ROWS):
    """Per-core out [128, ntiles] -> full [TOTAL_ROWS] f32."""
    outs = []
    for r in res_list:
        buf = np.asarray(r["out"])  # [128, ntiles]
        outs.append(buf.T.reshape(-1))  # row t*128+p = buf[p, t]
    return np.concatenate(outs)


def run(inputs, trace=False, mm_dt=MM_DT):
    """Run on the 8 NeuronCores. Returns (full_output [B,NR] f32, results)."""
    from concourse.bass_utils import run_bass_kernel_spmd

    in_maps = prep_host(**inputs, rows=ROWS, mm_dt=mm_dt)
    nc = get_program(ROWS, mm_dt)
    res = run_bass_kernel_spmd(nc, in_maps, list(range(NCORES)), trace=trace)
    out = unshard_out(res.results, ROWS)
    return out.reshape(B, NR).astype(np.float32), res


def kernel(**inputs):
    out, _ = run(inputs, trace=False)
    return out


# revision 39
# speedup vs baseline: 2.0922x; 1.5231x over previous
"""Trainium2 Bass kernel for nn_BiLinAntisymmetricFunc.

Math: out[b,n] = g(x1[b,n]) - g(x2[b,n]) + sum_k alpha_k * x1^T (U_k V_k^T - V_k U_k^T) x2

The bilinear part collapses: with S = sum_k alpha_k (U_k V_k^T - V_k U_k^T)
(a precomputable [D,D] weight matrix), bili = x1^T S x2 per row. Using
antisymmetry, bili = rowsum(x1 . (x2 @ (-S))): ONE [N,D]@[D,D] matmul.

Sharding: data-parallel over the 65536 rows (B*NR) -> 8 cores x 8192 rows.

v3 design:
  - all x tensors in bf16 (half HBM traffic, full PE rate, FWL weight loads)
  - one packed DMA per 512-row pair (x1t | x2t | x1row concatenated: 3MB)
    -> ~17 large transfers per core instead of ~112 (DMA fixed cost is ~2us
    per serialized HWDGE transfer on this part)
  - h1 matmuls 2-way column-tiled (tile_position via out base partition 0/32
    for x1, 64/96 for x2) -> 2 concurrent 512-col streams; all 16 chunk
    matmuls accumulate into ONE PSUM bank using per-element has_written
    (single start=True on the first matmul)
  - h-chain software-pipelined one pair ahead: PE order per steady block p is
    [P-matmuls pair p] [h2 pair p+1] [h1 pair p+2] [g pair p+1], so ACT/DVE
    round trips never stall the PE
  - g(x1)-g(x2) via h2stack [16,512] and w3stack [W3;-W3]: one [16,128]x[16,1]
    matmul per 128-row tile into a shared [128,4] PSUM tile
  - P PSUM tile is [128,1024] (2 banks); one DVE scalar_tensor_tensor per
    128-row tile does x1.P with rowsum accum -> [128,1]
  - outputs collect into a [128,64] SBUF buffer, one 32KB store per core;
    out[p,t] = row t*128+p (host transposes)
"""

import os

import numpy as np

D, K, RANK = 1024, 8, 64
B, NR = 16, 4096
NCORES = 8
TOTAL_ROWS = B * NR
ROWS = TOTAL_ROWS // NCORES  # 8192 rows per core

MM_DT = os.environ.get("BILIN_MM_DT", "bf16")

_PROG_CACHE = {}

NCHUNK = D // 128  # 8 contraction chunks
PAIR = 512  # rows per group (four 128-row tiles)
NTILE = PAIR // 128
XT_COLS = NCHUNK * PAIR  # 4096
XROW_COLS = NTILE * D  # 4096
XALL_COLS = 2 * XT_COLS + XROW_COLS  # x1t | x2t | x1row


def _build_program(rows, mm_dt, variant=None, reps=1):
    # variant switches: "dmaonly" (skip compute), "computeonly" (no per-pair DMA)
    variant = variant if variant is not None else os.environ.get("BILIN_VARIANT", "")
    dmaonly = "dmaonly" in variant
    computeonly = "computeonly" in variant
    import concourse.bacc as bacc
    import concourse.mybir as mybir
    import concourse.tile as tile

    f32 = mybir.dt.float32
    bf16 = mybir.dt.bfloat16
    if mm_dt == "bf16":
        mdt = bf16
    else:
        mdt = mybir.dt.float32r

    nc = bacc.Bacc("TRN2", target_bir_lowering=False, debug=False)

    npairs = rows // PAIR
    ntiles = rows // 128

    xall_d = nc.dram_tensor("xall", [npairs, 128, XALL_COLS], mdt, kind="ExternalInput")
    s_d = nc.dram_tensor("s", [128, NCHUNK * D], mdt, kind="ExternalInput")  # packed -S
    w1_d = nc.dram_tensor("w1", [128, NCHUNK * K], mdt, kind="ExternalInput")
    b1_d = nc.dram_tensor("b1", [K, 1], f32, kind="ExternalInput")
    w2_d = nc.dram_tensor("w2", [K, K], mdt, kind="ExternalInput")
    b2_d = nc.dram_tensor("b2", [K, 1], f32, kind="ExternalInput")
    w3f_d = nc.dram_tensor("w3f", [40, 1], mdt, kind="ExternalInput")  # [W3;0*24;-W3]
    # out[p, t] = result row t*128+p  (host transposes)
    out_d = nc.dram_tensor("out", [128, ntiles], f32, kind="ExternalOutput")

    relu = mybir.ActivationFunctionType.Relu
    mult = mybir.AluOpType.mult

    with tile.TileContext(nc) as tc:
        with (
            tc.tile_pool(name="const", bufs=1) as cpool,
            tc.tile_pool(name="xall", bufs=4) as xpool,
            tc.tile_pool(name="hsb", bufs=2) as hpool,
            tc.tile_pool(name="scr", bufs=2) as scrpool,
            tc.tile_pool(name="acc", bufs=8) as accpool,
            tc.tile_pool(name="pp", bufs=3, space="PSUM") as ppool,
            tc.tile_pool(name="hp1a", bufs=1, space="PSUM") as hp1apool,
            tc.tile_pool(name="hp1b", bufs=1, space="PSUM") as hp1bpool,
            tc.tile_pool(name="hp2", bufs=1, space="PSUM") as hp2pool,
            tc.tile_pool(name="gp", bufs=2, space="PSUM") as gppool,
        ):
            # ---- resident constants ----
            s_sb = cpool.tile([128, NCHUNK * D], mdt)  # chunk c at cols [c*D,(c+1)*D)
            nc.sync.dma_start(s_sb[:], s_d[:, :])
            w1_sb = cpool.tile([128, NCHUNK * K], mdt)  # chunk c at cols [c*K,(c+1)*K)
            nc.sync.dma_start(w1_sb[:], w1_d[:, :])
            w2_sb = cpool.tile([K, K], mdt)
            nc.sync.dma_start(w2_sb[:], w2_d[:, :])
            w3f_sb = cpool.tile([40, 1], mdt)
            nc.sync.dma_start(w3f_sb[:], w3f_d[:, :])
            b1_sb = cpool.tile([K, 1], f32)
            nc.sync.dma_start(b1_sb[:], b1_d[:, :])
            # persistent ping-pong h2 stacks; rows 8-31 must stay zero (they
            # multiply w3f's zero rows in the g matmul; garbage could be NaN)
            h2stack_tiles = [cpool.tile([40, PAIR], mdt, name=f"h2s40_{i}") for i in range(2)]
            for t_ in h2stack_tiles:
                nc.vector.memset(t_[:], 0.0)
            b2_sb = cpool.tile([K, 1], f32)
            nc.sync.dma_start(b2_sb[:], b2_d[:, :])
            outbuf = cpool.tile([128, ntiles], f32, name="outbuf")
            if dmaonly:
                nc.vector.memset(outbuf[:], 0.0)

            # pipeline state (trace-time): pair index -> live tile objects
            xall = {}
            h1sb = {}
            h2stack = {}
            gps = {}

            pre_xall = None
            if computeonly:
                pre_xall = cpool.tile([128, XALL_COLS], mdt, name="pre_xall")
                nc.sync.dma_start(pre_xall[:], xall_d[0, :, :])

            if computeonly:
                for j in range(npairs):
                    xall[j] = pre_xall

            def load(j, eng=None):
                if computeonly:
                    return
                t = xpool.tile([128, XALL_COLS], mdt, tag="xall")
                (eng or nc.sync).dma_start(t[:], xall_d[j % npairs, :, :])
                xall[j % npairs] = t

            def x1t(j):
                return xall[j][:, 0:XT_COLS]

            def x2t(j):
                return xall[j][:, XT_COLS : 2 * XT_COLS]

            def x1row(j):
                return xall[j][:, 2 * XT_COLS :]

            def emit_h1(j):
                """h1^T for both x of pair j: 16 chunk-matmuls, 2-way
                col-tiled across two PSUM banks (x1 @ base 0 of bank A, x2 @
                base 64 of bank B; interleaved for stream concurrency). Each
                bank has its own clean accumulation group — a shared bank
                would race its bank-wide has_written clear against the
                concurrent other group's first writes.
                ACT relus (one aligned PSUM read each) -> h1sb [8,512] per x."""
                jj = j % npairs
                h1pa = hp1apool.tile([128, PAIR], f32, tag="h1pa", name=f"h1pa{j}")
                h1pb = hp1bpool.tile([128, PAIR], f32, tag="h1pb", name=f"h1pb{j}")
                parts = (h1pa, h1pb)
                xts = (x1t(jj), x2t(jj))
                for c in range(NCHUNK):
                    for xi in range(2):
                        grp = 64 * xi
                        nc.tensor.matmul(
                            parts[xi][grp : grp + K, :],
                            w1_sb[:, c * K : (c + 1) * K],
                            xts[xi][:, c * PAIR : (c + 1) * PAIR],
                            start=(c == 0),
                            stop=(c == NCHUNK - 1),
                            tile_position=(0, grp),
                        )
                # two separate base-0 tiles: the h2 matmul contracts over
                # partitions, so rhs must share lhsT's (w2) partition range
                hs1 = hpool.tile([K, PAIR], mdt, tag="h1sb", name=f"h1sb{j}_0")
                hs2 = hpool.tile([K, PAIR], mdt, tag="h1sb", name=f"h1sb{j}_1")
                for xi, hs in enumerate((hs1, hs2)):
                    nc.scalar.activation(
                        hs[:, :], parts[xi][64 * xi : 64 * xi + K, :], relu, bias=b1_sb[:]
                    )
                h1sb[j % npairs] = (hs1, hs2)

            def emit_h2(j):
                """h2^T for pair j: 2 matmuls into the two bf16 column-halves
                of ONE bank (single-write groups, start=True each — safe under
                scheduler reordering since values survive the other group's
                bank-bit clear). ACT relus -> the [40,512] h2 stack (x1 rows
                0-7, x2 rows 32-39; rows 8-31 stay zero from the one-time
                memset) so g needs just one matmul per tile."""
                jj = j % npairs
                hs1, hs2 = h1sb[jj]
                h2ps = hp2pool.tile([128, PAIR], f32, tag="h2ps", name=f"h2p{j}")
                nc.tensor.matmul(
                    h2ps[0:K, :], w2_sb[:], hs1[:, :], start=True, stop=True,
                    skip_group_check=True,
                )
                nc.tensor.matmul(
                    h2ps[32 : 32 + K, :], w2_sb[:], hs2[:, :], start=True, stop=True,
                    skip_group_check=True, tile_position=(0, 32),
                )
                h2s40 = h2stack_tiles[j % 2]
                nc.scalar.activation(h2s40[0:K, :], h2ps[0:K, :], relu, bias=b2_sb[:])
                nc.scalar.activation(
                    h2s40[32 : 32 + K, :], h2ps[32 : 32 + K, :], relu, bias=b2_sb[:]
                )
                h2stack[jj] = h2s40

            def emit_g(j):
                """gdiff for pair j: per 128-row tile, one [40,128]x[40,1]
                matmul (rows 8-31 of the stack are zero, w3f is [W3;0;-W3])
                single-writing one column of a [128,4] PSUM tile."""
                jj = j % npairs
                h2s40 = h2stack[jj]
                g = gppool.tile([128, NTILE], f32, tag="g", name=f"g{j}")
                for i in range(NTILE):
                    nc.tensor.matmul(
                        g[:, i : i + 1],
                        h2s40[:, i * 128 : (i + 1) * 128],
                        w3f_sb[:],
                        start=True,
                        stop=True,
                        skip_group_check=True,
                    )
                gps[jj] = g

            def emit_P(p):
                """P matmuls + rowsum TTRs for all 4 tiles of pair p.
                Seg-major order: seg0's 8-chunk accumulation completes (and
                its TTR can run) while seg1's matmuls stream — so 3 PSUM
                banks (bufs=3) suffice for cross-tile double buffering."""
                pp = p % npairs
                for i in range(NTILE):
                    psegs = [
                        ppool.tile([128, 512], f32, tag="P", name=f"P{p}_{i}_{s_}")
                        for s_ in range(2)
                    ]
                    for seg in range(2):
                        for c in range(NCHUNK):
                            lhs = x2t(pp)[:, c * PAIR + i * 128 : c * PAIR + (i + 1) * 128]
                            nc.tensor.matmul(
                                psegs[seg][:],
                                lhs,
                                s_sb[:, c * D + seg * 512 : c * D + (seg + 1) * 512],
                                start=(c == 0),
                                stop=(c == NCHUNK - 1),
                            )
                    acc0 = accpool.tile([128, 1], f32, tag="acc")
                    acc1 = accpool.tile([128, 1], f32, tag="acc")
                    acc01 = accpool.tile([128, 1], f32, tag="acc")
                    scr0 = scrpool.tile([128, 512], f32, tag="scr")
                    scr1 = scrpool.tile([128, 512], f32, tag="scr")
                    nc.vector.scalar_tensor_tensor(
                        scr0[:], x1row(pp)[:, i * D : i * D + 512], 1.0, psegs[0][:],
                        op0=mult, op1=mult, accum_out=acc0[:],
                    )
                    nc.vector.scalar_tensor_tensor(
                        scr1[:], x1row(pp)[:, i * D + 512 : (i + 1) * D], 1.0, psegs[1][:],
                        op0=mult, op1=mult, accum_out=acc1[:],
                    )
                    nc.vector.tensor_add(acc01[:], acc0[:], acc1[:])
                    accs.append((p, i, acc01))

            def emit_final(p):
                pp = p % npairs
                g = gps[pp]
                for (q, i, acc) in [a for a in accs if a[0] == p]:
                    nc.vector.tensor_add(
                        outbuf[:, pp * NTILE + i : pp * NTILE + i + 1],
                        acc[:],
                        g[:, i : i + 1],
                    )
                accs[:] = [a for a in accs if a[0] != p]

            accs = []

            PREFETCH = 3

            def emit_full():
                """One complete self-contained pass: prologue (first loads +
                h-chain for pairs 0,1, g for pair 0) then the pair blocks."""
                for j in range(min(PREFETCH, npairs)):
                    load(j)
                if not dmaonly:
                    emit_h1(0)
                    emit_h2(0)
                    emit_h1(1)
                    emit_g(0)
                for p in range(npairs):
                    jn = p + PREFETCH
                    if jn < npairs:
                        load(jn)
                    if dmaonly:
                        continue
                    emit_P(p)
                    # final(p) before g(p+1): gps is single-buffered, so the
                    # g matmuls of p+1 must WAR-wait on final(p)'s reads
                    emit_final(p)
                    if p + 1 < npairs:
                        emit_h2(p + 1)
                    if p + 2 < npairs:
                        emit_h1(p + 2)
                    if p + 1 < npairs:
                        emit_g(p + 1)
                nc.sync.dma_start(out_d[:, :], outbuf[:])

            if reps > 1:
                with tc.For_i(0, reps, 1):
                    emit_full()
            else:
                emit_full()
    nc.compile()
    return nc


def get_program(rows=ROWS, mm_dt=MM_DT):
    key = (rows, mm_dt)
    if key not in _PROG_CACHE:
        _PROG_CACHE[key] = _build_program(rows, mm_dt)
    return _PROG_CACHE[key]


def _pack_xt(x, npairs):
    """[rows, D] -> [npairs, 128, NCHUNK*PAIR]; [pair,p,c*PAIR+r] = x[pair*PAIR+r, c*128+p]."""
    return (
        x.reshape(npairs, PAIR, NCHUNK, 128)
        .transpose(0, 3, 2, 1)
        .reshape(npairs, 128, NCHUNK * PAIR)
    )


def _pack_xrow(x, npairs):
    """[rows, D] -> [npairs, 128, NTILE*D]; [pair,p,i*D+d] = x[pair*PAIR+i*128+p, d]."""
    return (
        x.reshape(npairs, NTILE, 128, D).transpose(0, 2, 1, 3).reshape(
            npairs, 128, NTILE * D
        )
    )


def prep_host(x1, x2, U, V, alpha, W1, b1, W2, b2, W3, b3, rows=ROWS, mm_dt=MM_DT):
    """Host-side prep: fold U,V,alpha into -S, stack W3, shard + pack x."""
    f64 = np.float64
    Uf = np.asarray(U, f64).transpose(1, 0, 2).reshape(D, K * RANK)
    Vaf = (np.asarray(V, f64) * np.asarray(alpha, f64)[:, None, None])
    Vaf = Vaf.transpose(1, 0, 2).reshape(D, K * RANK)
    A = Uf @ Vaf.T
    s_use = (A.T - A)  # == -S ; bili = rowsum(x1 * (x2 @ s_use))

    import ml_dtypes

    mnp = np.dtype(ml_dtypes.bfloat16) if mm_dt == "bf16" else np.dtype(np.float32)

    npairs = rows // PAIR
    # pack [D, N] -> [128, NCHUNK*N]: row c*128+p -> [p, c*N + n]
    s_pk = np.ascontiguousarray(
        s_use.reshape(NCHUNK, 128, D).transpose(1, 0, 2).reshape(128, NCHUNK * D)
    ).astype(mnp)
    w1 = np.ascontiguousarray(
        np.asarray(W1, np.float64).reshape(NCHUNK, 128, K).transpose(1, 0, 2).reshape(128, NCHUNK * K)
    ).astype(mnp)
    w2 = np.ascontiguousarray(np.asarray(W2, np.float32)).astype(mnp)
    w3f = np.concatenate(
        [np.asarray(W3, f64), np.zeros((24, 1), f64), -np.asarray(W3, f64)], axis=0
    ).astype(mnp)
    b1c = np.ascontiguousarray(np.asarray(b1, np.float32).reshape(K, 1))
    b2c = np.ascontiguousarray(np.asarray(b2, np.float32).reshape(K, 1))

    x1f = np.asarray(x1, np.float32).reshape(TOTAL_ROWS, D)
    x2f = np.asarray(x2, np.float32).reshape(TOTAL_ROWS, D)
    ncores = TOTAL_ROWS // rows
    in_maps = []
    for c in range(ncores):
        sl = slice(c * rows, (c + 1) * rows)
        xall = np.concatenate(
            [
                _pack_xt(x1f[sl], npairs),
                _pack_xt(x2f[sl], npairs),
                _pack_xrow(x1f[sl], npairs),
            ],
            axis=2,
        ).astype(mnp)
        in_maps.append(
            {
                "xall": np.ascontiguousarray(xall),
                "s": s_pk,
                "w1": w1,
                "b1": b1c,
                "w2": w2,
                "b2": b2c,
                "w3f": w3f,
            }
        )
    return in_maps


def unshard_out(res_list, rows=ROWS):
    """Per-core out [128, ntiles] -> full [TOTAL_ROWS] f32."""
    outs = []
    for r in res_list:
        buf = np.asarray(r["out"])  # [128, ntiles]
        outs.append(buf.T.reshape(-1))  # row t*128+p = buf[p, t]
    return np.concatenate(outs)


def run(inputs, trace=False, mm_dt=MM_DT):
    """Run on the 8 NeuronCores. Returns (full_output [B,NR] f32, results)."""
    from concourse.bass_utils import run_bass_kernel_spmd

    in_maps = prep_host(**inputs, rows=ROWS, mm_dt=mm_dt)
    nc = get_program(ROWS, mm_dt)
    res = run_bass_kernel_spmd(nc, in_maps, list(range(NCORES)), trace=trace)
    out = unshard_out(res.results, ROWS)
    return out.reshape(B, NR).astype(np.float32), res


def kernel(**inputs):
    out, _ = run(inputs, trace=False)
    return out
